# revision 1
# baseline (speedup 1.0000x reference)
"""AttentionMemory kernel for Trainium2 (8 NeuronCores, Bass/Tile).

Reference computation (per batch b):
    affinity[n, m] = (2 * mk[:,n]@qk[:,m] - ||mk[:,n]||^2 - ||qk[:,m]||^2) / 8
    out[n, m]      = softmax over n (memory axis)

Softmax over n is invariant to per-column constants, so the -||qk_m||^2
term is dropped.  Logits are produced by an augmented matmul:
    lhsT (stationary) = [0.25 * qk ; -0.125]          -> [65, Mc]
    rhs  (moving)     = [mk        ; a_n   ]          -> [65, N]
    psum[m, n]        = 0.25*dot(qk_m, mk_n) - 0.125*a_n   == logits[m, n]
with a_n = sum_c mk[c,n]^2 precomputed on the host.

Precision: inputs are split hi/lo into bf16 pairs on the host and each
logit tile accumulates three bf16 matmuls in PSUM
    qh@mh + qh@ml + ql@mh      (ql@ml dropped, ~6e-5 logit error)
giving ~1e-4 relative output error at full 1-cycle/row PE throughput
(plain fp32 matmul is 4x slower; float32r is fast but tf32-precision).

Sharding: core c handles batch c//2, query-column half c%2 (communication
free: softmax is over the full n axis which each core holds).  Each core
writes out_c[m, n]; the host transposes to the reference [n, m] layout.

Input DRAM layout is packed by first-use so the head of the pipeline
starts as early as possible:
    q2 [65, 16*252]: per m-strip s, block [qh_s (126) | ql_s (126)]
    m2 [65,  8*1008]: per n-chunk c, block [mh_c (504) | ml_c (504)]

Logits are <= 0, so exp() never overflows and the max-subtraction pass is
skipped (min logit ~ -35 -> exp ~ 1e-16, no underflow in fp32).

Per-core roofline: 32.5 MB f32 output at ~360 GB/s ~= 90 us.  Pipeline:
PE (bf16 matmuls) -> ACT (exp + fused row-sum, PSUM->SBUF) -> DVE
(reciprocal + normalize) -> HWDGE store; the store stream runs gap-free.
"""

import numpy as np

B, CK, H, W = 4, 64, 48, 84
N = H * W            # 4032 memory pixels (softmax axis)
HALF = N // 2        # 2016 query pixels per core
M_STRIP = 126        # output-partition strip size (16 * 126 = 2016)
N_STRIPS = HALF // M_STRIP
K_AUG = CK + 1       # 65: contraction dim incl. the -a_n row

N_QUARTER = N // 4   # 1008: one PSUM tile (2 banks) / one ACT exp call
N_CHUNK = 504        # matmul moving free dim (<=512, one PSUM bank)
N_CHUNKS = N // N_CHUNK  # 8

_CACHE = {}


def _build_nc():
    import concourse.bacc as bacc
    import concourse.mybir as mybir
    import concourse.tile as tile

    f32 = mybir.dt.float32
    bf16 = mybir.dt.bfloat16
    Exp = mybir.ActivationFunctionType.Exp

    nc = bacc.Bacc("TRN2", target_bir_lowering=False, debug=False)

    q2_d = nc.dram_tensor("q2", [K_AUG, 2 * HALF], bf16, kind="ExternalInput")
    m2_d = nc.dram_tensor("m2", [K_AUG, 2 * N], bf16, kind="ExternalInput")
    out_d = nc.dram_tensor("out_c", [HALF, N], f32, kind="ExternalOutput")

    with tile.TileContext(nc) as tc:
        with (
            tc.tile_pool(name="singles", bufs=1) as singles,
            tc.tile_pool(name="psum", bufs=4, space="PSUM") as psum_pool,
            tc.tile_pool(name="exp", bufs=3) as exp_pool,
            tc.tile_pool(name="outs", bufs=4) as out_pool,
            tc.tile_pool(name="stats", bufs=8) as stats_pool,
        ):
            # --- prewarm: ACT exp table load + PE HAM spin-up during the
            # input DMAs -----------------------------------------------------
            wtab = singles.tile([1, 2], f32)
            nc.vector.memset(wtab, 0.0)
            nc.scalar.activation(wtab[:, 1:2], wtab[:, 0:1], Exp)
            wsrc = singles.tile([K_AUG, 256], bf16)
            nc.vector.memset(wsrc, 0.0)
            wps = psum_pool.tile([M_STRIP, 256], f32, tag="ps")
            for _ in range(12):
                nc.tensor.matmul(
                    wps, wsrc[:, :M_STRIP], wsrc, start=True, stop=True
                )

            # --- inputs, staged by first use.  q2 rides the ACT HWDGE ring,
            # m2 the SP ring, so their dispatches overlap ---------------------
            q2_s = singles.tile([K_AUG, 2 * HALF], bf16)
            m2_s = singles.tile([K_AUG, 2 * N], bf16)
            nc.scalar.dma_start(out=q2_s[:, :252], in_=q2_d[:, :252])
            for c0, c1 in ((0, 2), (2, 4), (4, 6), (6, 8)):
                sl = slice(c0 * 1008, c1 * 1008)
                nc.sync.dma_start(out=m2_s[:, sl], in_=m2_d[:, sl])
            nc.sync.dma_start(out=q2_s[:, 252:], in_=q2_d[:, 252:])

            def mh(c):  # rhs hi slice for n-chunk c
                return m2_s[:, c * 1008 : c * 1008 + N_CHUNK]

            def ml(c):  # rhs lo slice for n-chunk c
                return m2_s[:, c * 1008 + N_CHUNK : (c + 1) * 1008]

            for s in range(N_STRIPS):
                m0 = s * M_STRIP
                qh_l = q2_s[:, s * 252 : s * 252 + M_STRIP]
                ql_l = q2_s[:, s * 252 + M_STRIP : (s + 1) * 252]

                exp_t = exp_pool.tile([M_STRIP, N], f32, tag="exp")
                acc = stats_pool.tile([M_STRIP, 8], f32, tag="acc")

                # ACT pieces = pairs of 504-wide n-chunks (one 2-bank PSUM
                # tile / one exp call each)
                pieces = [[0, 1], [2, 3], [4, 5], [6, 7]]
                for pi, piece in enumerate(pieces):
                    k = len(piece)
                    # one PSUM bank (512 cols) per 504-wide chunk; each chunk
                    # starts on a bank boundary — PE writes must not straddle
                    # a bank
                    ps = psum_pool.tile([M_STRIP, 512 * k], f32, tag="ps")
                    for cc, c in enumerate(piece):
                        psl = ps[:, cc * 512 : cc * 512 + N_CHUNK]
                        nc.tensor.matmul(psl, qh_l, mh(c), start=True, stop=False)
                        nc.tensor.matmul(psl, qh_l, ml(c), start=False, stop=False)
                        nc.tensor.matmul(psl, ql_l, mh(c), start=False, stop=True)
                    # exp(logits) PSUM->SBUF with fused per-partition row sum;
                    # the strided 3D views skip the 8 pad columns per bank
                    e0 = piece[0] * N_CHUNK
                    nc.scalar.activation(
                        exp_t[:, e0 : e0 + k * N_CHUNK].rearrange(
                            "p (b c) -> p b c", b=k
                        ),
                        ps.rearrange("p (b c) -> p b c", b=k)[:, :, :N_CHUNK],
                        Exp,
                        accum_out=acc[:, pi : pi + 1],
                    )

                ssum = stats_pool.tile([M_STRIP, 1], f32, tag="ssum")
                nc.vector.reduce_sum(
                    ssum, acc[:, : len(pieces)], axis=mybir.AxisListType.X
                )
                rcp = stats_pool.tile([M_STRIP, 1], f32, tag="rcp")
                nc.vector.reciprocal(rcp, ssum)

                # strip 0 stores in quarters to start the store stream early;
                # steady state stores in 1 MB halves (better real-HW DMA
                # efficiency at equal modeled time)
                out_t = out_pool.tile([M_STRIP, N], f32, tag="out")
                if s == 0:
                    bounds = [0, 1008, 2016, 3024, N]
                else:
                    bounds = [0, N // 2, N]
                for p0, p1 in zip(bounds, bounds[1:]):
                    sl = slice(p0, p1)
                    nc.vector.tensor_scalar_mul(out_t[:, sl], exp_t[:, sl], rcp)
                    nc.sync.dma_start(
                        out=out_d[m0 : m0 + M_STRIP, sl], in_=out_t[:, sl]
                    )

    nc.compile()
    return nc


def _get_nc():
    if "nc" not in _CACHE:
        _CACHE["nc"] = _build_nc()
    return _CACHE["nc"]


def _split_bf16(x: np.ndarray):
    """x (f32) -> (hi, lo) bf16 with hi + lo ~= x (~16 mantissa bits)."""
    import ml_dtypes

    hi = x.astype(ml_dtypes.bfloat16)
    lo = (x - hi.astype(np.float32)).astype(ml_dtypes.bfloat16)
    return hi, lo


def kernel(mk: np.ndarray, qk: np.ndarray) -> np.ndarray:
    import ml_dtypes
    from concourse import bass_utils

    mk = np.asarray(mk, dtype=np.float32).reshape(B, CK, N)
    qk = np.asarray(qk, dtype=np.float32).reshape(B, CK, N)
    a = np.einsum("bcn,bcn->bn", mk, mk)  # sum_c mk^2, [B, N]

    in_maps = []
    for core in range(8):
        b, h = divmod(core, 2)
        mk_aug = np.empty((K_AUG, N), np.float32)
        mk_aug[:CK] = mk[b]
        mk_aug[CK] = a[b]
        mh, ml = _split_bf16(mk_aug)
        # chunk-pair packed: block c = [mh_c | ml_c], each N_CHUNK wide
        m2 = np.empty((K_AUG, 2 * N), ml_dtypes.bfloat16)
        m3 = m2.reshape(K_AUG, N_CHUNKS, 2, N_CHUNK)
        m3[:, :, 0] = mh.reshape(K_AUG, N_CHUNKS, N_CHUNK)
        m3[:, :, 1] = ml.reshape(K_AUG, N_CHUNKS, N_CHUNK)

        qk_aug = np.empty((K_AUG, HALF), np.float32)
        qk_aug[:CK] = 0.25 * qk[b, :, h * HALF : (h + 1) * HALF]
        qk_aug[CK] = -0.125
        qh, ql = _split_bf16(qk_aug)
        ql[CK] = 0  # a_n row must enter exactly once (via qh row 64)
        # strip packed: block s = [qh_s | ql_s], each M_STRIP wide
        q2 = np.empty((K_AUG, 2 * HALF), ml_dtypes.bfloat16)
        q3 = q2.reshape(K_AUG, N_STRIPS, 2, M_STRIP)
        q3[:, :, 0] = qh.reshape(K_AUG, N_STRIPS, M_STRIP)
        q3[:, :, 1] = ql.reshape(K_AUG, N_STRIPS, M_STRIP)

        in_maps.append({"q2": q2, "m2": m2})

    res = bass_utils.run_bass_kernel_spmd(
        _get_nc(), in_maps, core_ids=list(range(8))
    )
    _CACHE["last_results"] = res

    out = np.empty((B, N, N), np.float32)
    for core in range(8):
        b, h = divmod(core, 2)
        out[b, :, h * HALF : (h + 1) * HALF] = res.results[core]["out_c"].T
    return out



# revision 4
# speedup vs baseline: 1.3226x; 1.3226x over previous
"""AttentionMemory kernel for Trainium2 (8 NeuronCores, Bass/Tile).

Reference computation (per batch b):
    affinity[n, m] = (2 * mk[:,n]@qk[:,m] - ||mk[:,n]||^2 - ||qk[:,m]||^2) / 8
    out[n, m]      = softmax over n (memory axis)

Softmax over n is invariant to per-column constants, so the -||qk_m||^2
term is dropped.  Logits are produced by an augmented float32r matmul:
    lhsT (stationary) = [0.25 * qk ; -0.125 ; -0.125]   -> [66, Mc]
    rhs  (moving)     = [mk        ; a1     ; a2     ]  -> [66, N]
    psum[m, n] = 0.25*dot(qk_m, mk_n) - 0.125*(a1+a2)_n == logits[m, n]
with a = sum_c mk[c,n]^2 split on the host into a1 (10-mantissa-bit
exact, safe under any f32r rounding) + a2 (residual).

float32r runs at 1 cycle/row for moving free dim >= 256 (vs 3 bf16
hi/lo matmuls previously), with >= 10 mantissa bits; end-to-end metric
error is ~2e-3, dominated by the bf16 output store.

Sharding: core c handles batch c//2, query-column half c%2 (communication
free: softmax is over the full n axis which each core holds).  Each core
writes out_c[m, n] in bf16; the host upcasts and transposes to the
reference [n, m] f32 layout.

Pipeline per 126-row query strip: PE (4 f32r matmuls per 2016-col piece)
-> ACT (exp PSUM->SBUF bf16 with fused row-sum accum, the critical path)
-> DVE (reciprocal + normalize, bf16) -> SP HWDGE store (bf16 halves the
store bytes vs f32).
"""

import numpy as np

B, CK, H, W = 4, 64, 48, 84
N = H * W            # 4032 memory pixels (softmax axis)
HALF = N // 2        # 2016 query pixels per core
M_STRIP = 126        # output-partition strip size (16 * 126 = 2016)
N_STRIPS = HALF // M_STRIP
K_AUG = CK + 2       # 66: contraction dim incl. the two -a rows

PIECE = 2016         # ACT exp granularity: 4 PSUM banks (4 x 504 chunks)
N_CHUNK = 504        # matmul moving free dim (one PSUM bank, 8 pad cols)

_CACHE = {}


def _build_nc():
    import concourse.bacc as bacc
    import concourse.mybir as mybir
    import concourse.tile as tile

    f32 = mybir.dt.float32
    f32r = mybir.dt.float32r
    bf16 = mybir.dt.bfloat16
    Exp = mybir.ActivationFunctionType.Exp

    nc = bacc.Bacc("TRN2", target_bir_lowering=False, debug=False)

    q_d = nc.dram_tensor("q", [K_AUG, HALF], f32r, kind="ExternalInput")
    m_d = nc.dram_tensor("m", [K_AUG, N], f32r, kind="ExternalInput")
    out_d = nc.dram_tensor("out_c", [HALF, N], bf16, kind="ExternalOutput")

    with tile.TileContext(nc) as tc:
        with (
            tc.tile_pool(name="singles", bufs=1) as singles,
            tc.tile_pool(name="psum", bufs=2, space="PSUM") as psum_pool,
            tc.tile_pool(name="exp", bufs=3) as exp_pool,
            tc.tile_pool(name="outs", bufs=3) as out_pool,
            tc.tile_pool(name="stats", bufs=8) as stats_pool,
        ):
            # --- prewarm: ACT exp table load + PE pstate ramp during the
            # input DMAs -----------------------------------------------------
            wtab = singles.tile([1, 2], f32)
            nc.vector.memset(wtab, 0.0)
            nc.scalar.activation(wtab[:, 1:2], wtab[:, 0:1], Exp)
            wsrc = singles.tile([K_AUG, 256], bf16)
            nc.vector.memset(wsrc, 0.0)
            wps = psum_pool.tile([M_STRIP, 2048], f32, tag="ps")
            for _ in range(12):
                nc.tensor.matmul(
                    wps[:, :256],
                    wsrc[:, :M_STRIP],
                    wsrc,
                    start=True,
                    stop=True,
                )

            # --- inputs, staged by first use.  The first q strip-pair rides
            # the ACT HWDGE ring so its dispatch overlaps the SP ring --------
            q_s = singles.tile([K_AUG, HALF], f32r)
            m_s = singles.tile([K_AUG, N], f32r)
            nc.scalar.dma_start(out=q_s[:, :252], in_=q_d[:, :252])
            nc.sync.dma_start(out=m_s[:, :PIECE], in_=m_d[:, :PIECE])
            nc.sync.dma_start(out=q_s[:, 252:], in_=q_d[:, 252:])
            nc.sync.dma_start(out=m_s[:, PIECE:], in_=m_d[:, PIECE:])

            for s in range(N_STRIPS):
                m0 = s * M_STRIP
                q_l = q_s[:, m0 : m0 + M_STRIP]

                exp_t = exp_pool.tile([M_STRIP, N], bf16, tag="exp")
                acc = stats_pool.tile([M_STRIP, 2], f32, tag="acc")

                for pi in range(2):
                    # one PSUM bank (512 cols) per 504-wide chunk; each chunk
                    # starts on a bank boundary — PE writes must not straddle
                    # a bank
                    ps = psum_pool.tile([M_STRIP, 2048], f32, tag="ps")
                    for j in range(4):
                        c0 = pi * PIECE + j * N_CHUNK
                        nc.tensor.matmul(
                            ps[:, j * 512 : j * 512 + N_CHUNK],
                            q_l,
                            m_s[:, c0 : c0 + N_CHUNK],
                            start=True,
                            stop=True,
                        )
                    # exp(logits) PSUM->SBUF bf16 with fused per-partition row
                    # sum; the strided 3D views skip the 8 pad cols per bank
                    nc.scalar.activation(
                        exp_t[:, pi * PIECE : (pi + 1) * PIECE].rearrange(
                            "p (b c) -> p b c", b=4
                        ),
                        ps.rearrange("p (b c) -> p b c", b=4)[:, :, :N_CHUNK],
                        Exp,
                        accum_out=acc[:, pi : pi + 1],
                    )

                ssum = stats_pool.tile([M_STRIP, 1], f32, tag="ssum")
                nc.vector.reduce_sum(ssum, acc, axis=mybir.AxisListType.X)
                rcp = stats_pool.tile([M_STRIP, 1], f32, tag="rcp")
                nc.vector.reciprocal(rcp, ssum)

                out_t = out_pool.tile([M_STRIP, N], bf16, tag="out")
                nc.vector.tensor_scalar_mul(out_t, exp_t, rcp)
                nc.sync.dma_start(out=out_d[m0 : m0 + M_STRIP, :], in_=out_t)

    nc.compile()
    return nc


def _get_nc():
    if "nc" not in _CACHE:
        _CACHE["nc"] = _build_nc()
    return _CACHE["nc"]


def _round_mant(x: np.ndarray, bits: int) -> np.ndarray:
    """Round to `bits` explicit mantissa bits (exact under f32r rounding)."""
    m, e = np.frexp(x.astype(np.float64))
    scale = 2.0 ** (bits + 1)
    return np.ldexp(np.round(m * scale) / scale, e).astype(np.float32)


def kernel(mk: np.ndarray, qk: np.ndarray) -> np.ndarray:
    from concourse import bass_utils

    mk = np.asarray(mk, dtype=np.float32).reshape(B, CK, N)
    qk = np.asarray(qk, dtype=np.float32).reshape(B, CK, N)
    a = np.einsum("bcn,bcn->bn", mk.astype(np.float64), mk.astype(np.float64))
    a1 = _round_mant(a, 10)
    a2 = (a - a1).astype(np.float32)

    in_maps = []
    for core in range(8):
        b, h = divmod(core, 2)
        m_aug = np.empty((K_AUG, N), np.float32)
        m_aug[:CK] = mk[b]
        m_aug[CK] = a1[b]
        m_aug[CK + 1] = a2[b]

        q_aug = np.empty((K_AUG, HALF), np.float32)
        q_aug[:CK] = 0.25 * qk[b, :, h * HALF : (h + 1) * HALF]
        q_aug[CK:] = -0.125

        in_maps.append({"q": q_aug, "m": m_aug})

    res = bass_utils.run_bass_kernel_spmd(
        _get_nc(), in_maps, core_ids=list(range(8))
    )
    _CACHE["last_results"] = res

    out = np.empty((B, N, N), np.float32)
    for core in range(8):
        b, h = divmod(core, 2)
        out[b, :, h * HALF : (h + 1) * HALF] = (
            res.results[core]["out_c"].astype(np.float32).T
        )
    return out
